# revision 1
# baseline (speedup 1.0000x reference)
"""Bahdanau additive attention (causal, masked) on 8 Trainium2 NeuronCores.

Reference computation (B=4, S=512, D=256, U=256), fp32:
    q = values @ Wq ; v = values @ Wv
    score[b,i,j] = sum_u Vw[u] * tanh(q[b,i,u] + v[b,j,u])  (+ causal & key masks)
    attn = softmax(score, axis=-1)
    context = (attn @ values) * query_mask

Sharding: 8 cores = (batch b in 0..3) x (query-parity h in 0..1). Core (b,h)
handles batch b and the 256 queries {i : i % 2 == h}. Parity interleaving makes
the causal work profile identical across cores, so a single SPMD program works
for all 8 — all per-core differences (query gather, causal mask, key mask) are
input data, not program structure.

Per-core device program (engine balance: ACT ~125us tanh is the floor;
PE score matmuls ~100us and DVE adds ~70us hide under it):
  - project values to qT[u,i] (fp32 out) / vproj[u,j] (fp16) with fp16
    matmuls; a small "bootstrap" projection (288 keys x 16 queries) unblocks
    the first tanh batches ~5us earlier than the full-width projections
  - per query i: DVE tensor_scalar_add (fp16, 4x mode) computes
    vproj + qT[:,i] into a 16-query batch tile; one ACT Tanh instruction
    covers the whole batch (in-place)
  - score rows via PE matmuls with one-hot Vw weights (lhsT = Vw x e_i in
    32-wide strips, tile_position pinning the PSUM row strip) accumulating
    into a [128,512] PSUM score tile initialized by a K=1 ones x key-mask
    matmul (start=True)
  - causal mask via DVE add of a per-core constant; softmax: DVE max,
    ACT exp with accum_out sum, DVE reciprocal
  - context: PE transpose of attn, PE matmul against values (fp16), scale
    by 1/sum and query mask, DMA out
  - causal work is balanced across cores by query-parity interleaving; the
    per-query key extent (JEXT) is identical across cores, so one SPMD
    program serves all 8
"""

import sys

sys.path.insert(0, "/opt/trn_rl_repo")

import numpy as np

import concourse.bass as bass
import concourse.bacc as bacc
import concourse.tile as tile
from concourse import mybir
from concourse.bass_utils import run_bass_kernel_spmd

B, S, D, U = 4, 512, 256, 256
N_CORES = 8
NEG16 = -30000.0  # additive mask value (fp16-safe; exp() underflows to 0 like -1e9)

f32 = mybir.dt.float32
f16 = mybir.dt.float16
u8 = mybir.dt.uint8
AF = mybir.ActivationFunctionType
AX = mybir.AxisListType


def _jext_table():
    """Causal key extent per local query slot k (identical for both parities).

    Local slot k in [0,256): block = k//128, pos = k%128, global query
    g_h = 256*block + 2*pos + h.  Extent covers max(g_0, g_1)+1 keys,
    rounded up to 32.
    """
    je = []
    for k in range(256):
        blk, p = divmod(k, 128)
        need = 256 * blk + 2 * p + 2  # = g_{h=1} + 1 >= g_{h=0} + 1
        je.append(min(S, 32 * ((need + 31) // 32)))
    return je


JEXT = _jext_table()


def _build_program():
    nc = bacc.Bacc("TRN2", target_bir_lowering=False, debug=False)

    values_ap = nc.dram_tensor("values", [S, D], f16, kind="ExternalInput").ap()
    valsT_ap = nc.dram_tensor("valuesT", [D, S], f16, kind="ExternalInput").ap()
    valqT_ap = nc.dram_tensor("valqT", [D, 256], f16, kind="ExternalInput").ap()
    wq_ap = nc.dram_tensor("wq", [D, U], f16, kind="ExternalInput").ap()
    wv_ap = nc.dram_tensor("wv", [D, U], f16, kind="ExternalInput").ap()
    voh_ap = nc.dram_tensor("voh", [U, 1024], f16, kind="ExternalInput").ap()
    causal_ap = nc.dram_tensor("causal", [256, S], f16, kind="ExternalInput").ap()
    qm_ap = nc.dram_tensor("qm", [1, 256], f32, kind="ExternalInput").ap()
    msk_ap = nc.dram_tensor("mask_u8", [1, S], u8, kind="ExternalInput").ap()
    id32_ap = nc.dram_tensor("ident32", [128, 128], f32, kind="ExternalInput").ap()
    id16_ap = nc.dram_tensor("ident16", [128, 128], f16, kind="ExternalInput").ap()
    ctx_ap = nc.dram_tensor("ctx", [256, D], f32, kind="ExternalOutput").ap()

    from contextlib import ExitStack

    with tile.TileContext(nc) as tc, ExitStack() as es:
        const = es.enter_context(tc.tile_pool(name="const", bufs=1))
        work = es.enter_context(tc.tile_pool(name="work", bufs=1))
        tpool = es.enter_context(tc.tile_pool(name="tanh", bufs=7))
        epool = es.enter_context(tc.tile_pool(name="esc", bufs=2))
        etpool = es.enter_context(tc.tile_pool(name="escT", bufs=6))
        spool = es.enter_context(tc.tile_pool(name="smalls", bufs=4))
        opool = es.enter_context(tc.tile_pool(name="out", bufs=2))
        pp = es.enter_context(tc.tile_pool(name="psum", bufs=2, space="PSUM"))

        # ---- loads, in critical-path order ----
        # chain to first tanh: vT16+wv16 -> bootstrap proj -> adds -> tanh
        vT_sb = [work.tile([128, S], f16, tag=f"vT{dt}", name=f"vT{dt}") for dt in range(2)]
        valqT_sb = [work.tile([128, 256], f16, tag=f"vqT{dt}", name=f"vqT{dt}") for dt in range(2)]
        wq_sb, wv_sb = [], []
        # split the critical loads across the SP and GPSIMD DMA queues: the
        # SP sequencer serializes dma_start issues (~0.6us each), so putting
        # every second tile on the idle GPSIMD queue halves the issue chain
        for dt in range(2):
            eng = nc.sync if dt == 0 else nc.gpsimd
            eng.dma_start(vT_sb[dt][:], valsT_ap[128 * dt : 128 * (dt + 1), :])
        for dt in range(2):
            t2 = work.tile([128, U], f16, tag=f"wv{dt}")
            (nc.sync if dt == 0 else nc.gpsimd).dma_start(
                t2[:], wv_ap[128 * dt : 128 * (dt + 1), :]
            )
            wv_sb.append(t2)
        for dt in range(2):
            (nc.sync if dt == 0 else nc.gpsimd).dma_start(
                valqT_sb[dt][:], valqT_ap[128 * dt : 128 * (dt + 1), :]
            )
        for dt in range(2):
            t1 = work.tile([128, U], f16, tag=f"wq{dt}")
            (nc.sync if dt == 0 else nc.gpsimd).dma_start(
                t1[:], wq_ap[128 * dt : 128 * (dt + 1), :]
            )
            wq_sb.append(t1)
        voh_sb = []
        for ut in range(2):
            t = const.tile([128, 1024], f16, tag=f"voh{ut}")
            nc.gpsimd.dma_start(t[:], voh_ap[128 * ut : 128 * (ut + 1), :])
            voh_sb.append(t)
        # small mask inputs (gate the PSUM-init matmul) next
        qm_sb = const.tile([1, 256], f32, tag="qm")
        nc.sync.dma_start(qm_sb[:], qm_ap[:])
        msku = const.tile([1, S], u8, tag="msku")
        nc.sync.dma_start(msku[:], msk_ap[:])
        ones16 = const.tile([1, 128], f16, tag="ones16")
        nc.vector.memset(ones16[:], 1.0)
        mneg16 = const.tile([1, S], f16, tag="mneg16")
        nc.scalar.activation(
            mneg16[:], msku[:], AF.Copy, scale=-NEG16, bias=NEG16
        )

        # bootstrap projections: just enough (288 keys x 16 queries of the
        # first block processed) for the first 4 tanh batches
        vproj_sb = [work.tile([128, S], f16, tag=f"vp{ut}", name=f"vp{ut}") for ut in range(2)]
        # scalar operand of tensor_scalar must be fp32
        qT_sb = [work.tile([128, 256], f32, tag=f"qT{ut}", name=f"qT{ut}") for ut in range(2)]
        BOOT_J, BOOT_Q0, BOOT_QN = 288, 128, 16
        vboot_sb = [work.tile([128, BOOT_J], f16, tag=f"vb{ut}", name=f"vb{ut}") for ut in range(2)]
        qboot_sb = [work.tile([128, BOOT_QN], f32, tag=f"qb{ut}", name=f"qb{ut}") for ut in range(2)]
        for ut in range(2):
            # per u-tile: both matmuls, then both copies back-to-back, so the
            # in-order DVE queue unblocks ut0's adds as early as possible
            psb = pp.tile([128, BOOT_J], f32, tag="tp", name=f"psb{ut}")
            for dt in range(2):
                nc.tensor.matmul(
                    psb[:],
                    lhsT=wv_sb[dt][:, 128 * ut : 128 * (ut + 1)],
                    rhs=vT_sb[dt][:, :BOOT_J],
                    start=(dt == 0),
                    stop=(dt == 1),
                )
            psq = pp.tile([128, BOOT_J], f32, tag="tp", name=f"psq{ut}")
            for dt in range(2):
                nc.tensor.matmul(
                    psq[:, :BOOT_QN],
                    lhsT=wq_sb[dt][:, 128 * ut : 128 * (ut + 1)],
                    rhs=valqT_sb[dt][:, BOOT_Q0 : BOOT_Q0 + BOOT_QN],
                    start=(dt == 0),
                    stop=(dt == 1),
                )
            nc.vector.tensor_copy(vboot_sb[ut][:], psb[:])
            nc.vector.tensor_copy(qboot_sb[ut][:], psq[:, :BOOT_QN])

        # ---- main ----
        # Phase 1 (heavy block first): tanh batches + score matmuls.
        # Phase 2: causal add + softmax + context, heavy block first so the
        # kernel tail is the light block. Keeping all DVE adds ahead of the
        # causal adds avoids head-of-line blocking on the in-order DVE queue.
        G = 16  # queries per tanh batch
        BLK_ORDER = [1, 0]


        def _late_prep():
            # full projections (consumed from batch 5 on) + aux loads
            for ut in range(2):
                ps = pp.tile([128, S], f32, tag="score", name=f"psv{ut}")
                for dt in range(2):
                    nc.tensor.matmul(
                        ps[:],
                        lhsT=wv_sb[dt][:, 128 * ut : 128 * (ut + 1)],
                        rhs=vT_sb[dt][:],
                        start=(dt == 0),
                        stop=(dt == 1),
                    )
                nc.vector.tensor_copy(vproj_sb[ut][:], ps[:])
                ps2 = pp.tile([128, S], f32, tag="score", name=f"psq2{ut}")
                for dt in range(2):
                    nc.tensor.matmul(
                        ps2[:, 0:256],
                        lhsT=wq_sb[dt][:, 128 * ut : 128 * (ut + 1)],
                        rhs=valqT_sb[dt][:],
                        start=(dt == 0),
                        stop=(dt == 1),
                    )
                nc.vector.tensor_copy(qT_sb[ut][:], ps2[:, 0:256])
            for t in range(4):
                v16 = work.tile([128, D], f16, tag=f"v16_{t}", name=f"v16_{t}")
                nc.sync.dma_start(v16[:], values_ap[128 * t : 128 * (t + 1), :])
                v16_sb.append(v16)
            i32_l = const.tile([128, 128], f32, tag="i32", name="i32_sb")
            nc.sync.dma_start(i32_l[:], id32_ap[:])
            i16_l = const.tile([128, 128], f16, tag="i16", name="i16_sb")
            nc.sync.dma_start(i16_l[:], id16_ap[:])
            for blk2 in range(2):
                t = const.tile([128, S], f16, tag=f"causal{blk2}", name=f"causal{blk2}")
                nc.sync.dma_start(t[:], causal_ap[128 * blk2 : 128 * (blk2 + 1), :])
                causal_sb.append(t)
            ident.extend([i32_l, i16_l])
            for blk2 in range(2):
                tpq = pp.tile([128, 128], f32, tag="tp", name=f"tpq{blk2}")
                nc.tensor.transpose(
                    tpq[:, 0:1],
                    qm_sb[0:1, 128 * blk2 : 128 * (blk2 + 1)],
                    i32_l[0:1, 0:1],
                )
                qc = spool.tile([128, 1], f32, tag="qmcol", name=f"qmcol{blk2}")
                nc.vector.tensor_copy(qc[:], tpq[:, 0:1])
                qmcol_sb.append(qc)

        v16_sb, causal_sb, ident, qmcol_sb = [], [], [], []

        score_tiles = {}
        for blk in BLK_ORDER:
            score = pp.tile([128, S], f32, tag="score", name=f"score{blk}")
            nc.tensor.matmul(
                score[:],
                lhsT=ones16[:],
                rhs=mneg16[:],
                start=True,
                stop=False,
                skip_group_check=True,
            )
            n_mm = 0
            if blk == BLK_ORDER[0]:
                batches = [(0, 4), (4, 4), (8, 4), (12, 4)] + [
                    (g, G) for g in range(16, 128, G)
                ]
            else:
                batches = [(g, G) for g in range(128 - G, -1, -G)]
            for bi, (g0, bsz) in enumerate(batches):
                if blk == BLK_ORDER[0] and bi == 4:
                    _late_prep()
                jeg = JEXT[128 * blk + g0 + bsz - 1]
                strip = g0 // 32
                boot = blk == BLK_ORDER[0] and g0 + bsz <= BOOT_QN
                if boot:
                    assert jeg <= BOOT_J and 128 * blk == BOOT_Q0
                for ut in range(2):
                    Tg = tpool.tile([128, G * S], f16, tag="T", name=f"T{blk}_{g0}_{ut}")
                    for gi in range(bsz):
                        p = g0 + gi
                        k = 128 * blk + p
                        nc.vector.tensor_scalar_add(
                            Tg[:, gi * jeg : gi * jeg + jeg],
                            vboot_sb[ut][:, :jeg] if boot else vproj_sb[ut][:, :jeg],
                            qboot_sb[ut][:, p : p + 1] if boot else qT_sb[ut][:, k : k + 1],
                        )
                    nc.scalar.activation(
                        Tg[:, : bsz * jeg], Tg[:, : bsz * jeg], AF.Tanh
                    )
                    for gi in range(bsz):
                        p = g0 + gi
                        k = 128 * blk + p
                        je = JEXT[k]
                        n_mm += 1
                        nc.tensor.matmul(
                            score[32 * strip : 32 * (strip + 1), :je],
                            lhsT=voh_sb[ut][:, 32 * (p % 32) : 32 * (p % 32 + 1)],
                            rhs=Tg[:, gi * jeg : gi * jeg + je],
                            start=False,
                            stop=(n_mm == 256),
                            skip_group_check=True,
                            tile_position=(0, 32 * strip),
                        )
            # causal mask (exact diagonal)
            nc.vector.tensor_add(score[:], score[:], causal_sb[blk][:])
            # softmax
            mx = spool.tile([128, 1], f32, tag="mx", name=f"mx{blk}")
            nc.vector.reduce_max(mx[:], score[:], axis=AX.X)
            negm = spool.tile([128, 1], f32, tag="negm", name=f"negm{blk}")
            nc.vector.tensor_scalar_mul(negm[:], mx[:], -1.0)
            esc = epool.tile([128, S], f16, tag="esc", name=f"esc{blk}")
            ssum = spool.tile([128, 1], f32, tag="ssum", name=f"ssum{blk}")
            nc.scalar.activation(
                esc[:], score[:], AF.Exp, bias=negm[:], accum_out=ssum[:]
            )
            rcp = spool.tile([128, 1], f32, tag="rcp", name=f"rcp{blk}")
            nc.vector.reciprocal(rcp[:], ssum[:])
            rq = spool.tile([128, 1], f32, tag="rq", name=f"rq{blk}")
            nc.vector.tensor_mul(rq[:], rcp[:], qmcol_sb[blk][:])
            escT = []
            for jt in range(4):
                tpx = pp.tile([128, 128], f16, tag="tp", name=f"tp{blk}_{jt}")
                nc.tensor.transpose(
                    tpx[:], esc[:, 128 * jt : 128 * (jt + 1)], ident[1][:]
                )
                et = etpool.tile([128, 128], f16, tag="escT", name=f"escT{blk}_{jt}")
                nc.vector.tensor_copy(et[:], tpx[:])
                escT.append(et)
            ctxp = pp.tile([128, D], f32, tag="ctx", name=f"ctx{blk}")
            for jt in range(4):
                nc.tensor.matmul(
                    ctxp[:],
                    lhsT=escT[jt][:],
                    rhs=v16_sb[jt][:],
                    start=(jt == 0),
                    stop=(jt == 3),
                )
            ctxs = opool.tile([128, D], f32, tag="ctxs", name=f"ctxs{blk}")
            nc.vector.tensor_scalar_mul(ctxs[:], ctxp[:], rq[:, 0:1])
            nc.sync.dma_start(ctx_ap[128 * blk : 128 * (blk + 1), :], ctxs[:])

    nc.compile()
    return nc


_NC_CACHE = {}


def _get_nc():
    if "nc" not in _NC_CACHE:
        _NC_CACHE["nc"] = _build_program()
    return _NC_CACHE["nc"]


def _qsel(h):
    return np.concatenate([np.arange(h, 256, 2), np.arange(256 + h, 512, 2)])


def build_in_maps(values, mask, Wq, Wv, Vw):
    values = np.asarray(values, dtype=np.float32)
    mask = np.asarray(mask)
    Wq = np.asarray(Wq, dtype=np.float32)
    Wv = np.asarray(Wv, dtype=np.float32)
    Vw = np.asarray(Vw, dtype=np.float32)

    # one-hot Vw blocks: voh[u, r*32 + m] = Vw[u] if m == r else 0
    voh = np.zeros((U, 1024), dtype=np.float16)
    idx = np.arange(32) * 32 + np.arange(32)
    voh[:, idx] = Vw.astype(np.float16)[:, None]
    ident32 = np.eye(128, dtype=np.float32)
    ident16 = np.eye(128, dtype=np.float16)
    jcol = np.arange(S)

    in_maps = []
    for c in range(N_CORES):
        b, h = divmod(c, 2)
        qs = _qsel(h)
        causal = ((jcol[None, :] > qs[:, None]) * NEG16).astype(np.float16)
        qmask = mask[b][qs].astype(np.float32).reshape(1, 256)
        in_maps.append(
            {
                "values": values[b].astype(np.float16),
                "valuesT": np.ascontiguousarray(values[b].T.astype(np.float16)),
                "valqT": np.ascontiguousarray(values[b][qs].T.astype(np.float16)),
                "wq": Wq.astype(np.float16),
                "wv": Wv.astype(np.float16),
                "voh": voh,
                "causal": causal,
                "qm": np.ascontiguousarray(qmask),
                "mask_u8": mask[b].astype(np.uint8)[None, :],
                "ident32": ident32,
                "ident16": ident16,
            }
        )
    return in_maps


def kernel(values, mask, Wq, Wv, Vw):
    nc = _get_nc()
    in_maps = build_in_maps(values, mask, Wq, Wv, Vw)
    res = run_bass_kernel_spmd(nc, in_maps, list(range(N_CORES)))

    out = np.empty((B, S, D), dtype=np.float32)
    for c in range(N_CORES):
        b, h = divmod(c, 2)
        out[b, _qsel(h)] = res.results[c]["ctx"]
    return out



# revision 7
# speedup vs baseline: 3.2800x; 3.2800x over previous
"""Bahdanau additive attention (causal, masked) on 8 Trainium2 NeuronCores.

Reference computation (B=4, S=512, D=256, U=256), fp32:
    q = values @ Wq ; v = values @ Wv
    score[b,i,j] = sum_u Vw[u] * tanh(q[b,i,u] + v[b,j,u])  (+ causal & key masks)
    attn = softmax(score, axis=-1)
    context = (attn @ values) * query_mask

Sharding: 8 cores = (batch b in 0..3) x (query-parity h in 0..1). Core (b,h)
handles batch b and the 256 queries {i : i % 2 == h}. Parity interleaving makes
the two blocks of 128 local queries cover global ranges [0,256) and [256,512),
so causal key extents (256 / 512) are identical across cores and a single SPMD
program serves all 8.

Score path: tanh is a function of the SUM q_i + v_j, so it factorizes through
Fourier modes: tanh(x) ~= sum_k b_k sin(w_k x) (K=5 fitted frequencies), and
    sin(w(q+v)) = sin(wq)cos(wv) + cos(wq)sin(wv).
Each (u, k, sin/cos) pair is one contraction row of a plain PE matmul:
    score[i,j] = sum_t A[t,i] * B[t,j],  t = (u, k, f),  |t| = 256*5*2 = 2560.

The HW ACT Sin table is only valid for |arg| <~ 3.3 rad, so arguments are
range-reduced on DVE in "turn" units: T = q*(w/2pi) (+0.25 for the cosine
half), N = fp16-round(T + 1536) (fp16 output rounding at 1536 has ulp=1, so
this stores round(T)+1536 -- DVE internal arithmetic is fp32), G = 1536 - N =
-round(T), T += G, then one big ACT instruction computes sin(2pi * T) per
(side, u-tile). k=1 slices skip reduction (|w1*q| < pi always).

Engine split: ACT ~16us (4 big Sin + proj copies + 2 exp), DVE ~24us
(range reduction chains + softmax tail), PE ~15us (40+40 score matmuls,
projections, transposes, context), Pool: coefficient folds + DMA issue.
"""

import sys

sys.path.insert(0, "/opt/trn_rl_repo")

import numpy as np

import concourse.bass as bass
import concourse.bacc as bacc
import concourse.tile as tile
from concourse import mybir
from concourse.bass_utils import run_bass_kernel_spmd

B, S, D, U = 4, 512, 256, 256
N_CORES = 8
NEG16 = -30000.0  # additive mask value (fp16-safe; exp() underflows to 0)

# tanh(x) ~= sum_k FB[k] * sin(FW[k] * x), weighted LS fit on |x| <= 9
FW = [0.30352995930335663, 0.9170894581629232, 1.5455935041277598,
      2.193095021198692, 3.085846913740901]
FB = [1.2281174637752421, 0.3120153445172501, 0.1118054759503472,
      0.045456416171574726, 0.017575155897304136]
K = len(FW)
NF = 2 * K  # feature slices per side: [k2s..k5s, k2c..k5c, k1s, k1c]
TWOPI = float(2 * np.pi)
MAGIC = 1536.0  # fp16 ulp == 1 on [1024, 2048): +MAGIC then fp16-store rounds

# slice order: reduced slices first (contiguous region for the N/G/add pass),
# k=1 (never needs reduction) last.  SLICES[c] = (k, is_cos)
SLICES = [(k, 0) for k in range(1, K)] + [(k, 1) for k in range(1, K)] + [(0, 0), (0, 1)]
NRED = 2 * (K - 1)  # number of reduced slices
# matmul pairing: sin(q)cos(v) + cos(q)sin(v): q-slice (k,f) pairs with v-slice (k,1-f)
PARTNER = [SLICES.index((k, 1 - f)) for (k, f) in SLICES]

f32 = mybir.dt.float32
f16 = mybir.dt.float16
AF = mybir.ActivationFunctionType
AX = mybir.AxisListType
ALU = mybir.AluOpType


def _build_program():
    nc = bacc.Bacc("TRN2", target_bir_lowering=False, debug=False)

    values_ap = nc.dram_tensor("values", [S, D], f16, kind="ExternalInput").ap()
    valsT_ap = nc.dram_tensor("valuesT", [D, S], f16, kind="ExternalInput").ap()
    valqT_ap = nc.dram_tensor("valqT", [D, 256], f16, kind="ExternalInput").ap()
    wq_ap = nc.dram_tensor("wq", [D, U], f16, kind="ExternalInput").ap()
    wv_ap = nc.dram_tensor("wv", [D, U], f16, kind="ExternalInput").ap()
    causal_ap = nc.dram_tensor("causal", [256, S], f16, kind="ExternalInput").ap()
    vwb_ap = nc.dram_tensor("vwb", [128, 2 * NF], f32, kind="ExternalInput").ap()
    qmcol_ap = nc.dram_tensor("qmcol", [128, 2], f32, kind="ExternalInput").ap()
    id16_ap = nc.dram_tensor("ident16", [128, 128], f16, kind="ExternalInput").ap()
    ctx_ap = nc.dram_tensor("ctx", [256, D], f32, kind="ExternalOutput").ap()

    from contextlib import ExitStack

    with tile.TileContext(nc) as tc, ExitStack() as es:
        const = es.enter_context(tc.tile_pool(name="const", bufs=1))
        work = es.enter_context(tc.tile_pool(name="work", bufs=1))
        feat = es.enter_context(tc.tile_pool(name="feat", bufs=1))
        spool = es.enter_context(tc.tile_pool(name="smalls", bufs=4))
        epool = es.enter_context(tc.tile_pool(name="esc", bufs=2))
        etpool = es.enter_context(tc.tile_pool(name="escT", bufs=6))
        opool = es.enter_context(tc.tile_pool(name="out", bufs=2))
        pp = es.enter_context(tc.tile_pool(name="psum", bufs=2, space="PSUM"))

        # ---- loads, critical-path first. Alternate SP / GPSIMD DMA queues.
        vT_sb = [work.tile([128, S], f16, tag=f"vT{dt}", name=f"vT{dt}") for dt in range(2)]
        wv_sb, wq_sb, valqT_sb = [], [], []
        for dt in range(2):
            (nc.sync if dt == 0 else nc.gpsimd).dma_start(
                vT_sb[dt][:], valsT_ap[128 * dt : 128 * (dt + 1), :]
            )
        for dt in range(2):
            t2 = work.tile([128, U], f16, tag=f"wv{dt}")
            (nc.sync if dt == 0 else nc.gpsimd).dma_start(
                t2[:], wv_ap[128 * dt : 128 * (dt + 1), :]
            )
            wv_sb.append(t2)
        for dt in range(2):
            t1 = work.tile([128, 256], f16, tag=f"vqT{dt}")
            (nc.sync if dt == 0 else nc.gpsimd).dma_start(
                t1[:], valqT_ap[128 * dt : 128 * (dt + 1), :]
            )
            valqT_sb.append(t1)
        for dt in range(2):
            t1 = work.tile([128, U], f16, tag=f"wq{dt}")
            (nc.sync if dt == 0 else nc.gpsimd).dma_start(
                t1[:], wq_ap[128 * dt : 128 * (dt + 1), :]
            )
            wq_sb.append(t1)
        vwb_sb = const.tile([128, 2 * NF], f32, tag="vwb")
        nc.sync.dma_start(vwb_sb[:], vwb_ap[:])
        qmcol_sb = const.tile([128, 2], f32, tag="qmcol")
        nc.sync.dma_start(qmcol_sb[:], qmcol_ap[:])
        v16_sb = []
        for t in range(4):
            v16 = work.tile([128, D], f16, tag=f"v16_{t}", name=f"v16_{t}")
            (nc.sync if t % 2 == 0 else nc.gpsimd).dma_start(
                v16[:], values_ap[128 * t : 128 * (t + 1), :]
            )
            v16_sb.append(v16)
        causal_sb = []
        for blk in range(2):
            t = const.tile([128, S], f16, tag=f"causal{blk}", name=f"causal{blk}")
            (nc.sync if blk == 0 else nc.gpsimd).dma_start(
                t[:], causal_ap[128 * blk : 128 * (blk + 1), :]
            )
            causal_sb.append(t)
        id16_sb = const.tile([128, 128], f16, tag="i16", name="i16_sb")
        nc.gpsimd.dma_start(id16_sb[:], id16_ap[:])

        # ---- projections on PE; PSUM -> fp16 SBUF copies on ACT (Copy is in
        # every activation table, and ACT is idle while DVE builds arguments)
        vT16 = [work.tile([128, S], f16, tag=f"vp{ut}", name=f"vp{ut}") for ut in range(2)]
        qT16 = [work.tile([128, 256], f16, tag=f"qp{ut}", name=f"qp{ut}") for ut in range(2)]
        for ut in range(2):
            ps = pp.tile([128, S], f32, tag="proj", name=f"psv{ut}")
            for dt in range(2):
                nc.tensor.matmul(
                    ps[:],
                    lhsT=wv_sb[dt][:, 128 * ut : 128 * (ut + 1)],
                    rhs=vT_sb[dt][:],
                    start=(dt == 0),
                    stop=(dt == 1),
                )
            nc.scalar.activation(vT16[ut][:], ps[:], AF.Copy)
        for ut in range(2):
            ps = pp.tile([128, S], f32, tag="proj", name=f"psq{ut}")
            for dt in range(2):
                nc.tensor.matmul(
                    ps[:, 0:256],
                    lhsT=wq_sb[dt][:, 128 * ut : 128 * (ut + 1)],
                    rhs=valqT_sb[dt][:],
                    start=(dt == 0),
                    stop=(dt == 1),
                )
            nc.scalar.activation(qT16[ut][:], ps[:, 0:256], AF.Copy)

        # ---- range-reduced sin arguments in "turns" (arg/2pi), per (side,ut).
        # T slice c: q*(w/2pi) (+0.25 if cosine). For the first NRED slices:
        # N = fp16round(T+1536); G = 1536-N = -round(T); T += G.
        def build_args(src, n, ext):
            T = feat.tile([128, NF * ext], f16, tag=f"T{n}", name=f"T{n}")
            for c, (k, is_cos) in enumerate(SLICES):
                sl = T[:, c * ext : (c + 1) * ext]
                if is_cos:
                    nc.vector.tensor_scalar(
                        sl, src[:], FW[k] / TWOPI, 0.25, ALU.mult, ALU.add
                    )
                else:
                    nc.vector.tensor_scalar_mul(sl, src[:], FW[k] / TWOPI)
            red = T[:, 0 : NRED * ext]
            N = feat.tile([128, NRED * ext], f16, tag=f"N{n}", name=f"N{n}")
            nc.vector.tensor_scalar(N[:], red, MAGIC, None, ALU.add)
            nc.vector.tensor_scalar(N[:], N[:], -1.0, MAGIC, ALU.mult, ALU.add)
            nc.vector.tensor_add(red, red, N[:])
            return T

        # ---- features: one big ACT Sin (scale=2pi) per (side, ut)
        Bv, Aq = [], []
        for ut in range(2):
            T = build_args(vT16[ut], f"v{ut}", S)
            F = feat.tile([128, NF * S], f16, tag=f"Bv{ut}", name=f"Bv{ut}")
            nc.scalar.activation(F[:], T[:], AF.Sin, scale=TWOPI)
            Bv.append(F)
        for ut in range(2):
            T = build_args(qT16[ut], f"q{ut}", 256)
            F = feat.tile([128, NF * 256], f16, tag=f"Aq{ut}", name=f"Aq{ut}")
            nc.scalar.activation(F[:], T[:], AF.Sin, scale=TWOPI)
            Aq.append(F)
            # coefficient fold b_k * Vw[u] on the query side, in place.
            # ut0 on Pool (starts early, otherwise idle), ut1 on DVE (fast).
            eng = nc.gpsimd if ut == 0 else nc.vector
            for c, (k, _) in enumerate(SLICES):
                sl = F[:, c * 256 : (c + 1) * 256]
                eng.tensor_scalar_mul(sl, sl, vwb_sb[:, ut * NF + c : ut * NF + c + 1])

        # ---- score matmuls: block1 (512 keys) first, then block0 (256)
        JEXT = {1: 512, 0: 256}
        score = {}
        for blk in [1, 0]:
            ext = JEXT[blk]
            sc = pp.tile([128, ext], f32, tag="score", name=f"score{blk}")
            n = 0
            for ut in range(2):
                for c in range(NF):
                    p = PARTNER[c]
                    n += 1
                    nc.tensor.matmul(
                        sc[:],
                        lhsT=Aq[ut][:, c * 256 + 128 * blk : c * 256 + 128 * (blk + 1)],
                        rhs=Bv[ut][:, p * S : p * S + ext],
                        start=(n == 1),
                        stop=(n == 2 * NF),
                    )
            score[blk] = sc

        # ---- per block: causal mask + softmax + context
        for blk in [1, 0]:
            ext = JEXT[blk]
            sc = score[blk]
            nc.vector.tensor_add(sc[:], sc[:], causal_sb[blk][:, :ext])
            mx = spool.tile([128, 1], f32, tag="mx", name=f"mx{blk}")
            nc.vector.reduce_max(mx[:], sc[:], axis=AX.X)
            negm = spool.tile([128, 1], f32, tag="negm", name=f"negm{blk}")
            nc.vector.tensor_scalar_mul(negm[:], mx[:], -1.0)
            esc = epool.tile([128, ext], f16, tag="esc", name=f"esc{blk}")
            ssum = spool.tile([128, 1], f32, tag="ssum", name=f"ssum{blk}")
            nc.scalar.activation(esc[:], sc[:], AF.Exp, bias=negm[:], accum_out=ssum[:])
            rcp = spool.tile([128, 1], f32, tag="rcp", name=f"rcp{blk}")
            nc.vector.reciprocal(rcp[:], ssum[:])
            rq = spool.tile([128, 1], f32, tag="rq", name=f"rq{blk}")
            nc.vector.tensor_mul(rq[:], rcp[:], qmcol_sb[:, blk : blk + 1])
            escT = []
            for jt in range(ext // 128):
                tpx = pp.tile([128, 128], f16, tag="tp", name=f"tp{blk}_{jt}")
                nc.tensor.transpose(tpx[:], esc[:, 128 * jt : 128 * (jt + 1)], id16_sb[:])
                et = etpool.tile([128, 128], f16, tag="escT", name=f"escT{blk}_{jt}")
                nc.vector.tensor_copy(et[:], tpx[:])
                escT.append(et)
            ctxp = pp.tile([128, D], f32, tag="ctx", name=f"ctx{blk}")
            for jt in range(ext // 128):
                nc.tensor.matmul(
                    ctxp[:],
                    lhsT=escT[jt][:],
                    rhs=v16_sb[jt][:],
                    start=(jt == 0),
                    stop=(jt == ext // 128 - 1),
                )
            ctxs = opool.tile([128, D], f32, tag="ctxs", name=f"ctxs{blk}")
            nc.vector.tensor_scalar_mul(ctxs[:], ctxp[:], rq[:, 0:1])
            nc.sync.dma_start(ctx_ap[128 * blk : 128 * (blk + 1), :], ctxs[:])

    nc.compile()
    return nc


_NC_CACHE = {}


def _get_nc():
    if "nc" not in _NC_CACHE:
        _NC_CACHE["nc"] = _build_program()
    return _NC_CACHE["nc"]


def _qsel(h):
    return np.concatenate([np.arange(h, 256, 2), np.arange(256 + h, 512, 2)])


def build_in_maps(values, mask, Wq, Wv, Vw):
    values = np.asarray(values, dtype=np.float32)
    mask = np.asarray(mask)
    Wq = np.asarray(Wq, dtype=np.float32)
    Wv = np.asarray(Wv, dtype=np.float32)
    Vw = np.asarray(Vw, dtype=np.float32)

    ident16 = np.eye(128, dtype=np.float16)
    jcol = np.arange(S)
    # vwb[u, ut*NF + c] = FB[k(c)] * Vw[128*ut + u]
    fb_c = np.array([FB[k] for (k, _) in SLICES], dtype=np.float32)  # [NF]
    vwb = np.concatenate(
        [np.outer(Vw[:128], fb_c), np.outer(Vw[128:], fb_c)], axis=1
    ).astype(np.float32)

    kmask_add = ((1.0 - mask.astype(np.float32)) * NEG16).astype(np.float32)  # [B,S]

    in_maps = []
    for c in range(N_CORES):
        b, h = divmod(c, 2)
        qs = _qsel(h)
        causal = (jcol[None, :] > qs[:, None]) * NEG16 + kmask_add[b][None, :]
        causal = np.maximum(causal, NEG16).astype(np.float16)
        qmask = mask[b][qs].astype(np.float32)  # [256]
        qmcol = np.stack([qmask[:128], qmask[128:]], axis=1)  # [128, 2]
        in_maps.append(
            {
                "values": values[b].astype(np.float16),
                "valuesT": np.ascontiguousarray(values[b].T.astype(np.float16)),
                "valqT": np.ascontiguousarray(values[b][qs].T.astype(np.float16)),
                "wq": Wq.astype(np.float16),
                "wv": Wv.astype(np.float16),
                "causal": causal,
                "vwb": vwb,
                "qmcol": np.ascontiguousarray(qmcol),
                "ident16": ident16,
            }
        )
    return in_maps


def kernel(values, mask, Wq, Wv, Vw):
    nc = _get_nc()
    in_maps = build_in_maps(values, mask, Wq, Wv, Vw)
    res = run_bass_kernel_spmd(nc, in_maps, list(range(N_CORES)))

    out = np.empty((B, S, D), dtype=np.float32)
    for c in range(N_CORES):
        b, h = divmod(c, 2)
        out[b, _qsel(h)] = res.results[c]["ctx"]
    return out
